# revision 23
# baseline (speedup 1.0000x reference)
"""GMM negative log-likelihood on 8 TRN2 NeuronCores.

score[n, m] = wlog[m] - qf[n, m] factors exactly as F[n, :6] @ C[:6, m]
with features F = [1, x, y, x^2, xy, y^2].  The kernel computes scores
TRANSPOSED: each matmul produces a [128 component, 1024 sample] PSUM
tile (lhsT = a 128-column block of C, moving operand = the feature
chunk), so the mixture sum over m runs on the TensorEngine: after the
exp pass writes E = exp(score) to SBUF as bf16, eight [128,128]^T @
ones[128,1] matmuls per tile reduce over the component partitions with
a single moving column each, landing per-sample partial sums back in
PSUM with samples on partitions.  This removes both the activation
accumulator read-out and the vector-engine TensorReduce of the old
row-major design; ACT and DVE spend all their cycles on the exp pass.

The exp pass splits each 1024-sample tile between the scalar engine
(true Exp, psum f32 -> sbuf bf16) and the vector engine (Schraudolph
fast-exp: affine in f32, int16 cast-on-write = bf16 exponent/mantissa
bit construction) in ratio ~537:487 matching their throughputs.

Inputs arrive as one [102, 3072] bf16 blob: feature rows at partition
groups {0,32,64,96} (PE quadrant-aligned) with C replicated per group.
Data-parallel over N: each core gets 8192 samples and the full C.
Host sums the 8 component-block partials per sample and takes log in
f64.
"""

import numpy as np

import concourse.bacc as bacc
import concourse.bass as bass
import concourse.mybir as mybir
import concourse.tile as tile
from concourse.bass_utils import run_bass_kernel_spmd

N, M, NCORES = 65536, 1024, 8
NSH = N // NCORES          # 8192 samples per core
P = 128                    # partitions per tile
NG = 4                     # partition groups for features
GSH = NSH // NG            # 2048 samples per group
NCH = 2                    # sample chunks per group
CH = GSH // NCH            # 1024 samples per chunk
NMB = M // P               # 8 component blocks
BLOBW = M + GSH            # 3072 blob columns: [cmat | features]
ACOLS = 537                # sample columns on the scalar engine (true exp)
HALF = 512                 # psum bank width in f32 / max moving free dim
NRED = NG * NCH * NMB * (CH // P)   # 512 reduce partial columns

# Schraudolph in bf16: exp(s) ~= bitcast_bf16(int16(A*s + B)), A = 2^7/ln2.
# B = 2^7*(127 - c) with c making the relative error zero-mean over
# uniform mantissa fractions.
_SCH_A = float(2 ** 7 / np.log(2.0))
_SCH_C = float(np.log2(np.mean((1.0 + np.linspace(0, 1, 4097)) * 2.0 ** -np.linspace(0, 1, 4097))))
_SCH_B = float(2 ** 7 * (127.0 - _SCH_C))

_cache = {}


def _build(acols=ACOLS):
    f32 = mybir.dt.float32
    i16 = mybir.dt.int16
    bf16 = mybir.dt.bfloat16
    nc = bacc.Bacc(None, target_bir_lowering=False)

    blob_d = nc.declare_dram_parameter("blob", [102, BLOBW], bf16, isOutput=False)
    out_d = nc.declare_dram_parameter("out", [P, NRED], f32, isOutput=True)

    with tile.TileContext(nc) as tc:
        with (
            tc.tile_pool(name="const", bufs=1) as const,
            tc.tile_pool(name="ps", bufs=3, space=bass.MemorySpace.PSUM) as ps,
            tc.tile_pool(name="red", bufs=1, space=bass.MemorySpace.PSUM) as redp,
            tc.tile_pool(name="esb", bufs=8) as esb,
        ):
            blob = const.tile([102, BLOBW], bf16)
            # staged on two DGE queues in parallel: cmat (SP) and the first
            # feature chunk (DVE queue) land together so compute starts
            # early; the remaining chunk follows on SP.
            nc.sync.dma_start(out=blob[:, 0:M + CH], in_=blob_d[:, 0:M + CH])
            nc.sync.dma_start(out=blob[:, M + CH:BLOBW], in_=blob_d[:, M + CH:BLOBW])

            ones = const.tile([P, 1], bf16)
            nc.vector.memset(ones[:], 1.0)
            red = redp.tile([P, NRED], f32)  # one psum bank of partials

            # PE p-state warm-up: small data-independent matmuls keep the
            # tensor engine busy from t~0 so the real matmuls hit full clock
            # sooner; the target reuses a psum rotation slot that the main
            # loop only needs again ~3 tiles in.
            warm = const.tile([6, P], bf16)
            nc.vector.memset(warm[:], 0.0)
            wpt = ps.tile([P, CH], f32, tag="ps")
            for i in range(16):
                # alternate output columns: WAW acks overlap, PE stays busy
                # through the DMA lead-in so real matmuls start at full clock
                j = i % 8
                nc.tensor.matmul(wpt[:, j * P:(j + 1) * P], warm[:], warm[:],
                                 tile_position=(0, 0))

            # Reduce matmuls for a finished tile are emitted two iterations
            # late: by then the exp pass they depend on has completed, so
            # their semaphore waits are already satisfied at decode time and
            # they never clog the PE's 4-deep wait queue (which would stall
            # the sequencer and serialize the pipeline).
            pend = []

            def flush(entry):
                et, g, c, mb = entry
                base = ((c * NG + g) * NMB + mb) * (CH // P)
                for j in range(CH // P):
                    nc.tensor.matmul(
                        red[:, base + j:base + j + 1],
                        et[:, j * P:(j + 1) * P],
                        ones[:],
                        tile_position=(0, 0),
                    )

            for c in range(NCH):
                for g in range(NG):
                    gp = 32 * g
                    for mb in range(NMB):
                        lhsT = blob[gp:gp + 6, mb * P:(mb + 1) * P]
                        fmov = blob[gp:gp + 6, M + c * CH:M + (c + 1) * CH]
                        dcols = CH - acols
                        pt = ps.tile([P, CH], f32, tag="ps")
                        nc.tensor.matmul(pt[:, 0:HALF], lhsT, fmov[:, 0:HALF],
                                         tile_position=(gp, 0))
                        nc.tensor.matmul(pt[:, HALF:CH], lhsT, fmov[:, HALF:CH],
                                         tile_position=(gp, 0))
                        et = esb.tile([P, CH], bf16, tag="e")
                        # DVE takes the leading (slab-1-only) columns so its
                        # fast-exp can start as soon as the first matmul
                        # lands; ACT's true exp covers the tail.
                        nc.vector.tensor_scalar(
                            out=et[:, 0:dcols].bitcast(i16),
                            in0=pt[:, 0:dcols],
                            scalar1=_SCH_A, scalar2=_SCH_B,
                            op0=mybir.AluOpType.mult, op1=mybir.AluOpType.add,
                        )
                        nc.scalar.activation(
                            et[:, dcols:CH], pt[:, dcols:CH],
                            mybir.ActivationFunctionType.Exp,
                        )
                        if len(pend) >= 4:
                            flush(pend.pop(0))
                        pend.append((et, g, c, mb))
            while pend:
                flush(pend.pop(0))

            redsb = const.tile([P, NRED], f32)
            nc.scalar.copy(out=redsb[:, 0:NRED // 2], in_=red[:, 0:NRED // 2])
            nc.sync.dma_start(out=out_d[:, 0:NRED // 2], in_=redsb[:, 0:NRED // 2])
            nc.vector.tensor_copy(redsb[:, NRED // 2:NRED], red[:, NRED // 2:NRED])
            nc.sync.dma_start(out=out_d[:, NRED // 2:NRED],
                              in_=redsb[:, NRED // 2:NRED])

    nc.compile()
    return nc


def kernel(sample, mu, sigma_log, theta, w):
    import ml_dtypes

    x = sample[:, 0].astype(np.float64)
    y = sample[:, 1].astype(np.float64)
    mux = mu[:, 0].astype(np.float64)
    muy = mu[:, 1].astype(np.float64)
    sl = sigma_log.astype(np.float64)
    th = theta.astype(np.float64)
    wv = w[:, 0].astype(np.float64)

    a = np.exp(-2.0 * sl[:, 0])
    b = np.exp(-2.0 * sl[:, 1])
    c, s = np.cos(th), np.sin(th)
    g11 = a * c * c + b * s * s
    g12 = (a - b) * c * s
    g22 = a * s * s + b * c * c
    wmax = wv.max()
    wlog = (wv - (wmax + np.log(np.exp(wv - wmax).sum()))) - sl.sum(axis=1)

    # score = F @ C with F = [1, x, y, x^2, xy, y^2]
    cm = np.stack([
        wlog - (g11 * mux * mux + 2.0 * g12 * mux * muy + g22 * muy * muy),
        2.0 * (g11 * mux + g12 * muy),
        2.0 * (g12 * mux + g22 * muy),
        -g11,
        -2.0 * g12,
        -g22,
    ]).astype(np.float32)
    ftf = np.stack([np.ones_like(x), x, y, x * x, x * y, y * y]).astype(np.float32)

    cm16 = cm.astype(ml_dtypes.bfloat16)
    ftf16 = ftf.astype(ml_dtypes.bfloat16)

    if "nc" not in _cache:
        _cache["nc"] = _build()
    nc = _cache["nc"]

    in_maps = []
    for i in range(NCORES):
        blob = np.zeros((102, BLOBW), dtype=ml_dtypes.bfloat16)
        base = i * NSH
        for g in range(NG):
            gp = 32 * g
            blob[gp:gp + 6, 0:M] = cm16
            blob[gp:gp + 6, M:BLOBW] = ftf16[:, base + g * GSH:base + (g + 1) * GSH]
        in_maps.append({"blob": blob})
    res = run_bass_kernel_spmd(nc, in_maps, core_ids=list(range(NCORES)))
    _cache["last_result"] = res
    total = np.float64(0.0)
    for r in res.results:
        o = np.asarray(r["out"], dtype=np.float64)
        # columns ordered ((c, g, mb), j): sum the NMB partials per sample
        o = o.reshape(P, NCH * NG, NMB, CH // P)
        ssum = o.sum(axis=2)  # [P, g*c, j]
        total += np.log(ssum).sum()
    return np.float32(-total)


# revision 24
# speedup vs baseline: 1.0593x; 1.0593x over previous
"""GMM negative log-likelihood on 8 TRN2 NeuronCores.

score[n, m] = wlog[m] - qf[n, m] factors exactly as F[n, :6] @ C[:6, m]
with features F = [1, x, y, x^2, xy, y^2].  The kernel computes scores
TRANSPOSED: each matmul produces a [128 component, 1024 sample] PSUM
tile (lhsT = a 128-column block of C, moving operand = the feature
chunk), so the mixture sum over m runs on the TensorEngine: after the
exp pass writes E = exp(score) to SBUF as bf16, eight [128,128]^T @
ones[128,1] matmuls per tile reduce over the component partitions with
a single moving column each, landing per-sample partial sums back in
PSUM with samples on partitions.  This removes both the activation
accumulator read-out and the vector-engine TensorReduce of the old
row-major design; ACT and DVE spend all their cycles on the exp pass.

The exp pass splits each 1024-sample tile between the scalar engine
(true Exp, psum f32 -> sbuf bf16) and the vector engine (Schraudolph
fast-exp: affine in f32, int16 cast-on-write = bf16 exponent/mantissa
bit construction) in ratio ~537:487 matching their throughputs.

Inputs arrive as one [102, 3072] bf16 blob: feature rows at partition
groups {0,32,64,96} (PE quadrant-aligned) with C replicated per group.
Data-parallel over N: each core gets 8192 samples and the full C.
Host sums the 8 component-block partials per sample and takes log in
f64.
"""

import numpy as np

import concourse.bacc as bacc
import concourse.bass as bass
import concourse.mybir as mybir
import concourse.tile as tile
from concourse.bass_utils import run_bass_kernel_spmd

N, M, NCORES = 65536, 1024, 8
NSH = N // NCORES          # 8192 samples per core
P = 128                    # partitions per tile
NG = 4                     # partition groups for features
GSH = NSH // NG            # 2048 samples per group
NCH = 2                    # sample chunks per group
CH = GSH // NCH            # 1024 samples per chunk
NMB = M // P               # 8 component blocks
BLOBW = M + GSH            # 3072 blob columns: [cmat | features]
ACOLS = 537                # sample columns on the scalar engine (true exp)
HALF = 512                 # psum bank width in f32 / max moving free dim
NRED = NG * NCH * NMB * (CH // P)   # 512 reduce partial columns

# Schraudolph in bf16: exp(s) ~= bitcast_bf16(int16(A*s + B)), A = 2^7/ln2.
# B = 2^7*(127 - c) with c making the relative error zero-mean over
# uniform mantissa fractions.
_SCH_A = float(2 ** 7 / np.log(2.0))
_SCH_C = float(np.log2(np.mean((1.0 + np.linspace(0, 1, 4097)) * 2.0 ** -np.linspace(0, 1, 4097))))
_SCH_B = float(2 ** 7 * (127.0 - _SCH_C))

_cache = {}


def _build(acols=ACOLS):
    f32 = mybir.dt.float32
    i16 = mybir.dt.int16
    bf16 = mybir.dt.bfloat16
    nc = bacc.Bacc(None, target_bir_lowering=False)

    blob_d = nc.declare_dram_parameter("blob", [102, BLOBW], bf16, isOutput=False)
    out_d = nc.declare_dram_parameter("out", [P, NRED], f32, isOutput=True)

    with tile.TileContext(nc) as tc:
        with (
            tc.tile_pool(name="const", bufs=1) as const,
            tc.tile_pool(name="ps", bufs=3, space=bass.MemorySpace.PSUM) as ps,
            tc.tile_pool(name="red", bufs=1, space=bass.MemorySpace.PSUM) as redp,
            tc.tile_pool(name="esb", bufs=8) as esb,
        ):
            blob = const.tile([102, BLOBW], bf16)
            # staged on two DGE queues in parallel: cmat (SP) and the first
            # feature chunk (DVE queue) land together so compute starts
            # early; the remaining chunk follows on SP.
            nc.sync.dma_start(out=blob[:, 0:M + CH], in_=blob_d[:, 0:M + CH])
            nc.sync.dma_start(out=blob[:, M + CH:BLOBW], in_=blob_d[:, M + CH:BLOBW])

            ones = const.tile([P, 1], bf16)
            nc.vector.memset(ones[:], 1.0)
            red = redp.tile([P, NRED], f32)  # one psum bank of partials

            # PE p-state warm-up: small data-independent matmuls keep the
            # tensor engine busy from t~0 so the real matmuls hit full clock
            # sooner; the target reuses a psum rotation slot that the main
            # loop only needs again ~3 tiles in.
            warm = const.tile([6, P], bf16)
            nc.vector.memset(warm[:], 0.0)
            wpt = ps.tile([P, CH], f32, tag="ps")
            for i in range(16):
                # alternate output columns: WAW acks overlap, PE stays busy
                # through the DMA lead-in so real matmuls start at full clock
                j = i % 8
                nc.tensor.matmul(wpt[:, j * P:(j + 1) * P], warm[:], warm[:],
                                 tile_position=(0, 0))

            # Reduce matmuls for a finished tile are emitted two iterations
            # late: by then the exp pass they depend on has completed, so
            # their semaphore waits are already satisfied at decode time and
            # they never clog the PE's 4-deep wait queue (which would stall
            # the sequencer and serialize the pipeline).
            pend = []

            def flush(entry):
                et, g, c, mb = entry
                base = ((c * NG + g) * NMB + mb) * (CH // P)
                for j in range(CH // P):
                    nc.tensor.matmul(
                        red[:, base + j:base + j + 1],
                        et[:, j * P:(j + 1) * P],
                        ones[:],
                        tile_position=(0, 0),
                    )

            for c in range(NCH):
                for g in range(NG):
                    gp = 32 * g
                    for mb in range(NMB):
                        lhsT = blob[gp:gp + 6, mb * P:(mb + 1) * P]
                        fmov = blob[gp:gp + 6, M + c * CH:M + (c + 1) * CH]
                        dcols = CH - acols
                        pt = ps.tile([P, CH], f32, tag="ps")
                        nc.tensor.matmul(pt[:, 0:HALF], lhsT, fmov[:, 0:HALF],
                                         tile_position=(gp, 0))
                        nc.tensor.matmul(pt[:, HALF:CH], lhsT, fmov[:, HALF:CH],
                                         tile_position=(gp, 0))
                        et = esb.tile([P, CH], bf16, tag="e")
                        nc.scalar.activation(
                            et[:, 0:acols], pt[:, 0:acols],
                            mybir.ActivationFunctionType.Exp,
                        )
                        nc.vector.tensor_scalar(
                            out=et[:, acols:CH].bitcast(i16),
                            in0=pt[:, acols:CH],
                            scalar1=_SCH_A, scalar2=_SCH_B,
                            op0=mybir.AluOpType.mult, op1=mybir.AluOpType.add,
                        )
                        if len(pend) >= 4:
                            flush(pend.pop(0))
                        pend.append((et, g, c, mb))
            while pend:
                flush(pend.pop(0))

            redsb = const.tile([P, NRED], f32)
            nc.scalar.copy(out=redsb[:, 0:NRED // 2], in_=red[:, 0:NRED // 2])
            nc.sync.dma_start(out=out_d[:, 0:NRED // 2], in_=redsb[:, 0:NRED // 2])
            nc.vector.tensor_copy(redsb[:, NRED // 2:NRED], red[:, NRED // 2:NRED])
            nc.sync.dma_start(out=out_d[:, NRED // 2:NRED],
                              in_=redsb[:, NRED // 2:NRED])

    nc.compile()
    return nc


def kernel(sample, mu, sigma_log, theta, w):
    import ml_dtypes

    x = sample[:, 0].astype(np.float64)
    y = sample[:, 1].astype(np.float64)
    mux = mu[:, 0].astype(np.float64)
    muy = mu[:, 1].astype(np.float64)
    sl = sigma_log.astype(np.float64)
    th = theta.astype(np.float64)
    wv = w[:, 0].astype(np.float64)

    a = np.exp(-2.0 * sl[:, 0])
    b = np.exp(-2.0 * sl[:, 1])
    c, s = np.cos(th), np.sin(th)
    g11 = a * c * c + b * s * s
    g12 = (a - b) * c * s
    g22 = a * s * s + b * c * c
    wmax = wv.max()
    wlog = (wv - (wmax + np.log(np.exp(wv - wmax).sum()))) - sl.sum(axis=1)

    # score = F @ C with F = [1, x, y, x^2, xy, y^2]
    cm = np.stack([
        wlog - (g11 * mux * mux + 2.0 * g12 * mux * muy + g22 * muy * muy),
        2.0 * (g11 * mux + g12 * muy),
        2.0 * (g12 * mux + g22 * muy),
        -g11,
        -2.0 * g12,
        -g22,
    ]).astype(np.float32)
    ftf = np.stack([np.ones_like(x), x, y, x * x, x * y, y * y]).astype(np.float32)

    cm16 = cm.astype(ml_dtypes.bfloat16)
    ftf16 = ftf.astype(ml_dtypes.bfloat16)

    if "nc" not in _cache:
        _cache["nc"] = _build()
    nc = _cache["nc"]

    in_maps = []
    for i in range(NCORES):
        blob = np.zeros((102, BLOBW), dtype=ml_dtypes.bfloat16)
        base = i * NSH
        for g in range(NG):
            gp = 32 * g
            blob[gp:gp + 6, 0:M] = cm16
            blob[gp:gp + 6, M:BLOBW] = ftf16[:, base + g * GSH:base + (g + 1) * GSH]
        in_maps.append({"blob": blob})
    res = run_bass_kernel_spmd(nc, in_maps, core_ids=list(range(NCORES)))
    _cache["last_result"] = res
    total = np.float64(0.0)
    for r in res.results:
        o = np.asarray(r["out"], dtype=np.float64)
        # columns ordered ((c, g, mb), j): sum the NMB partials per sample
        o = o.reshape(P, NCH * NG, NMB, CH // P)
        ssum = o.sum(axis=2)  # [P, g*c, j]
        total += np.log(ssum).sum()
    return np.float32(-total)
